# revision 8
# baseline (speedup 1.0000x reference)
"""Grouped-expert SwiGLU MLP (MoE grouped GEMM) on 8 Trainium2 NeuronCores.

Strategy: expert-parallel SPMD. Core e owns expert e's weights and its
contiguous token block (tokens are pre-grouped by expert). All experts are
host-padded to a uniform tile count so a single SPMD program serves all 8
cores; per-core divergence lives entirely in the input data.

Per 512-token M-tile (on-chip, per core):
  xT   [128d x (16k x 512t)]   transposed load of x slice
  X1T_f/X3T_f (PSUM)[128f x 512t] = sum_k w{1,3}[k,f-chunk].T @ xT_k   (float32r)
  hT_f (SBUF) = silu(X1T_f) * X3T_f
  out[ts,dc] (PSUM)[128t x 512d] = sum_fc hT[fc,ts].T @ w2[fc,dc]      (float32r)
  out -> DRAM contiguous rows.
"""

import math
import os

import ml_dtypes
import numpy as np

D = 2048          # model dim
F = 512           # expert ffn dim
MT = 512          # tokens per M-tile
KC = D // 128     # 16 contraction chunks of 128
FC = F // 128     # 4 f chunks of 128
NCORES = 8

_cache = {}


def _build(pad_tiles: int):
    """Build + schedule the single-core SPMD program for pad_tiles M-tiles."""
    import concourse.bacc as bacc
    import concourse.mybir as mybir
    from concourse.tile import TileContext

    dt = mybir.dt
    f32 = dt.float32
    bf16 = dt.bfloat16
    PAD_T = pad_tiles * MT

    nc = bacc.Bacc(
        "TRN2",
        target_bir_lowering=False,
        debug=False,
        enable_asserts=False,
        num_devices=NCORES,
    )

    xpt = nc.dram_tensor("xpt", [D, PAD_T], bf16, kind="ExternalInput")
    w1 = nc.dram_tensor("w1", [D, F], bf16, kind="ExternalInput")
    w2 = nc.dram_tensor("w2", [F, D], bf16, kind="ExternalInput")
    w3 = nc.dram_tensor("w3", [D, F], bf16, kind="ExternalInput")
    out = nc.dram_tensor("out", [PAD_T, D], f32, kind="ExternalOutput")

    with TileContext(nc) as tc:
        with (
            tc.tile_pool(name="wpool", bufs=1) as wpool,
            tc.tile_pool(name="xt", bufs=6) as xt_pool,
            tc.tile_pool(name="ht", bufs=2) as ht_pool,
            tc.tile_pool(name="sil", bufs=3) as sil_pool,
            tc.tile_pool(name="osb", bufs=3) as osb_pool,
            tc.tile_pool(name="ps", bufs=8, space="PSUM") as ps_pool,
        ):
            # --- resident weights ---------------------------------------
            # w1/w3 stored [128d x (k,f)] : chunk (k) occupies free cols
            # [k*F, (k+1)*F); within it f-chunk at f*128.
            w1_sb = wpool.tile([128, KC, F], bf16, tag="w1")
            w3_sb = wpool.tile([128, KC, F], bf16, tag="w3")
            # w2 stored [128f x (fc,d)]: fc chunk at free cols [fc*D,(fc+1)*D)
            w2_sb = wpool.tile([128, FC, D], bf16, tag="w2")

            nc.sync.dma_start(
                out=w1_sb[:], in_=w1.ap().rearrange("(k p) f -> p k f", p=128)
            )
            nc.sync.dma_start(
                out=w3_sb[:], in_=w3.ap().rearrange("(k p) f -> p k f", p=128)
            )
            nc.sync.dma_start(
                out=w2_sb[:], in_=w2.ap().rearrange("(c p) d -> p c d", p=128)
            )

            for m in range(pad_tiles):
                # --- transposed x load: 4 tiles of 4 k-chunks each ------
                xts = []
                for g in range(4):
                    xt = xt_pool.tile([128, 4, MT], bf16, tag="xt")
                    for kk in range(4):
                        k = g * 4 + kk
                        nc.sync.dma_start(
                            out=xt[:, kk, :],
                            in_=xpt[k * 128:(k + 1) * 128, m * MT:(m + 1) * MT],
                        )
                    xts.append(xt)

                # --- GEMM1 + GEMM3 + SwiGLU, per f-chunk ----------------
                ht = ht_pool.tile([128, FC, MT], bf16, tag="ht")
                for f in range(FC):
                    x1t = ps_pool.tile([128, MT], f32, tag="ps")
                    x3t = ps_pool.tile([128, MT], f32, tag="ps")
                    for k in range(KC):
                        lhs1 = w1_sb[:, k, f * 128:(f + 1) * 128]
                        lhs3 = w3_sb[:, k, f * 128:(f + 1) * 128]
                        rhs = xts[k // 4][:, k % 4, :]
                        nc.tensor.matmul(
                            x1t[:], lhs1, rhs,
                            start=(k == 0), stop=(k == KC - 1),
                        )
                        nc.tensor.matmul(
                            x3t[:], lhs3, rhs,
                            start=(k == 0), stop=(k == KC - 1),
                        )
                    sig = sil_pool.tile([128, MT], f32, tag="sig")
                    nc.scalar.activation(
                        sig[:], x1t[:], mybir.ActivationFunctionType.Sigmoid
                    )
                    sil = sil_pool.tile([128, MT], f32, tag="sil")
                    nc.vector.tensor_mul(sil[:], x1t[:], sig[:])
                    nc.vector.tensor_mul(
                        ht[:, f, :], sil[:], x3t[:]
                    )

                # --- GEMM2: out[ts,dc] = sum_fc hT[fc,ts].T @ w2[fc,dc] -
                for ts in range(4):
                    osb = osb_pool.tile([128, D], f32, tag="osb")
                    for dc in range(4):
                        po = ps_pool.tile([128, 512], f32, tag="ps")
                        for fc in range(FC):
                            lhs = ht[:, fc, ts * 128:(ts + 1) * 128]
                            rhs = w2_sb[:, fc, dc * 512:(dc + 1) * 512]
                            nc.tensor.matmul(
                                po[:], lhs, rhs,
                                start=(fc == 0), stop=(fc == FC - 1),
                            )
                        nc.vector.tensor_copy(osb[:, dc * 512:(dc + 1) * 512], po[:])
                    nc.sync.dma_start(
                        out=out[m * MT + ts * 128: m * MT + (ts + 1) * 128, :],
                        in_=osb[:],
                    )

    nc.compile()
    return nc


def _get_program(pad_tiles: int):
    if pad_tiles not in _cache:
        _cache[pad_tiles] = _build(pad_tiles)
    return _cache[pad_tiles]


def kernel(x, num_tokens_per_expert, w1, w2, w3):
    from concourse.bass_utils import run_bass_kernel_spmd

    x = np.asarray(x)
    counts = np.asarray(num_tokens_per_expert).astype(np.int64)
    w1 = np.asarray(w1)
    w2 = np.asarray(w2)
    w3 = np.asarray(w3)

    T = x.shape[0]
    E = counts.shape[0]
    assert E == NCORES, f"expected {NCORES} experts, got {E}"
    starts = np.concatenate([[0], np.cumsum(counts)])[:E]

    pad_tiles = max(1, math.ceil(int(counts.max()) / MT))
    nc = _get_program(pad_tiles)
    PAD_T = pad_tiles * MT

    in_maps = []
    for e in range(E):
        cnt = int(counts[e])
        s = int(starts[e])
        xpt = np.zeros((D, PAD_T), dtype=ml_dtypes.bfloat16)
        xpt[:, :cnt] = x[s:s + cnt].T.astype(ml_dtypes.bfloat16)
        in_maps.append({
            "xpt": xpt,
            "w1": np.ascontiguousarray(w1[e]).astype(ml_dtypes.bfloat16),
            "w2": np.ascontiguousarray(w2[e]).astype(ml_dtypes.bfloat16),
            "w3": np.ascontiguousarray(w3[e]).astype(ml_dtypes.bfloat16),
        })

    trace = bool(int(os.environ.get("KERNEL_TRACE", "0")))
    try:
        res = run_bass_kernel_spmd(
            nc, in_maps, core_ids=list(range(NCORES)), trace=trace
        )
    except ModuleNotFoundError:
        res = run_bass_kernel_spmd(
            nc, in_maps, core_ids=list(range(NCORES)), trace=False
        )
    kernel.last_results = res

    out = np.empty((T, D), dtype=np.float32)
    for e in range(E):
        cnt = int(counts[e])
        s = int(starts[e])
        out[s:s + cnt] = res.results[e]["out"][:cnt]
    return out


# revision 10
# speedup vs baseline: 200.0856x; 200.0856x over previous
"""Balanced grouped-expert SwiGLU kernel: tokens tile-balanced across cores.

Each core executes NT=9 M-tile slots (vs 13 for expert-parallel padding).
A core's slots span at most 2 experts (A then B); the A/B switch index Ta
is per-core runtime data -> per-slot tc.If/Else picks resident weight set
wa or wb. Everything else is static.
"""

import math
import os

import ml_dtypes
import numpy as np

D = 2048
F = 512
MT = 512
KC = D // 128
FC = F // 128
NCORES = 8

_cache = {}


def _build(nt: int):
    import concourse.bacc as bacc
    import concourse.mybir as mybir
    from concourse.tile import TileContext

    dt = mybir.dt
    f32 = dt.float32
    bf16 = dt.bfloat16
    i32 = dt.int32
    PAD_T = nt * MT

    nc = bacc.Bacc(
        "TRN2", target_bir_lowering=False, debug=False,
        enable_asserts=False, num_devices=NCORES,
    )

    xpt = nc.dram_tensor("xpt", [D, PAD_T], bf16, kind="ExternalInput")
    wa1 = nc.dram_tensor("wa1", [D, F], bf16, kind="ExternalInput")
    wa2 = nc.dram_tensor("wa2", [F, D], bf16, kind="ExternalInput")
    wa3 = nc.dram_tensor("wa3", [D, F], bf16, kind="ExternalInput")
    wb1 = nc.dram_tensor("wb1", [D, F], bf16, kind="ExternalInput")
    wb2 = nc.dram_tensor("wb2", [F, D], bf16, kind="ExternalInput")
    wb3 = nc.dram_tensor("wb3", [D, F], bf16, kind="ExternalInput")
    meta = nc.dram_tensor("meta", [1, 1], i32, kind="ExternalInput")
    out = nc.dram_tensor("out", [PAD_T, D], f32, kind="ExternalOutput")

    with TileContext(nc) as tc:
        with (
            tc.tile_pool(name="wpool", bufs=1) as wpool,
            tc.tile_pool(name="xt", bufs=10) as xt_pool,
            tc.tile_pool(name="ht", bufs=2) as ht_pool,
            tc.tile_pool(name="sil", bufs=3) as sil_pool,
            tc.tile_pool(name="osb", bufs=4) as osb_pool,
            tc.tile_pool(name="ps", bufs=8, space="PSUM") as ps_pool,
        ):
            wsb = {}
            for pre, (t1, t2, t3) in (("a", (wa1, wa2, wa3)),
                                      ("b", (wb1, wb2, wb3))):
                s1 = wpool.tile([128, KC, F], bf16, tag=f"w1{pre}")
                s3 = wpool.tile([128, KC, F], bf16, tag=f"w3{pre}")
                s2 = wpool.tile([128, FC, D], bf16, tag=f"w2{pre}")
                nc.sync.dma_start(out=s1[:], in_=t1.ap().rearrange("(k p) f -> p k f", p=128))
                nc.sync.dma_start(out=s3[:], in_=t3.ap().rearrange("(k p) f -> p k f", p=128))
                nc.sync.dma_start(out=s2[:], in_=t2.ap().rearrange("(c p) d -> p c d", p=128))
                wsb[pre] = (s1, s2, s3)

            msb = wpool.tile([1, 1], i32, tag="meta")
            nc.sync.dma_start(out=msb[:], in_=meta.ap())
            ta_v = nc.snap(nc.values_load(msb[0:1, 0:1]))

            def tile_body(m, which):
                w1_sb, w2_sb, w3_sb = wsb[which]
                xts = []
                for g in range(4):
                    xt = xt_pool.tile([128, 4, MT], bf16, tag="xt")
                    for kk in range(4):
                        k = g * 4 + kk
                        nc.sync.dma_start(
                            out=xt[:, kk, :],
                            in_=xpt[k * 128:(k + 1) * 128, m * MT:(m + 1) * MT],
                        )
                    xts.append(xt)

                ht = ht_pool.tile([128, FC, MT], bf16, tag="ht")
                for f in range(FC):
                    x1t = ps_pool.tile([128, MT], f32, tag="ps")
                    x3t = ps_pool.tile([128, MT], f32, tag="ps")
                    for k in range(KC):
                        lhs1 = w1_sb[:, k, f * 128:(f + 1) * 128]
                        lhs3 = w3_sb[:, k, f * 128:(f + 1) * 128]
                        rhs = xts[k // 4][:, k % 4, :]
                        nc.tensor.matmul(x1t[:], lhs1, rhs,
                                         start=(k == 0), stop=(k == KC - 1))
                        nc.tensor.matmul(x3t[:], lhs3, rhs,
                                         start=(k == 0), stop=(k == KC - 1))
                    sig = sil_pool.tile([128, MT], f32, tag="sig")
                    nc.scalar.activation(sig[:], x1t[:],
                                         mybir.ActivationFunctionType.Sigmoid)
                    sil = sil_pool.tile([128, MT], f32, tag="sil")
                    nc.vector.tensor_mul(sil[:], x1t[:], sig[:])
                    nc.vector.tensor_mul(ht[:, f, :], sil[:], x3t[:])

                for ts in range(4):
                    osb = osb_pool.tile([128, D], f32, tag="osb")
                    for dc in range(4):
                        po = ps_pool.tile([128, 512], f32, tag="ps")
                        for fc in range(FC):
                            lhs = ht[:, fc, ts * 128:(ts + 1) * 128]
                            rhs = w2_sb[:, fc, dc * 512:(dc + 1) * 512]
                            nc.tensor.matmul(po[:], lhs, rhs,
                                             start=(fc == 0), stop=(fc == FC - 1))
                        nc.vector.tensor_copy(osb[:, dc * 512:(dc + 1) * 512], po[:])
                    nc.sync.dma_start(
                        out=out[m * MT + ts * 128: m * MT + (ts + 1) * 128, :],
                        in_=osb[:],
                    )

            for m in range(nt):
                with tc.If(ta_v > m) as cmp:
                    tile_body(m, "a")
                with cmp.Else():
                    tile_body(m, "b")

    nc.compile()
    return nc


def _get_program(nt: int):
    if nt not in _cache:
        _cache[nt] = _build(nt)
    return _cache[nt]


def _assign(counts):
    """Greedy: chunk the padded-tile list into per-core runs of <=NT tiles
    spanning <=2 experts. Returns (nt, per-core list of (expert, tile_lo,
    n_tiles) segment pairs) or None if infeasible."""
    E = len(counts)
    pt = [max(1, math.ceil(c / MT)) if c > 0 else 0 for c in counts]
    total = sum(pt)
    nt = math.ceil(total / NCORES)
    for nt_try in (nt, nt + 1):
        segs = [[] for _ in range(NCORES)]
        e, used = 0, 0
        ok = True
        for c in range(NCORES):
            cap = nt_try
            nexp = 0
            while cap > 0 and e < E:
                if pt[e] - used == 0:
                    e += 1
                    used = 0
                    continue
                if nexp == 2:
                    break
                take = min(cap, pt[e] - used)
                segs[c].append((e, used, take))
                used += take
                cap -= take
                nexp += 1
            # couldn't place everything and ran out of cores
        if e == E or (e == E - 1 and used == pt[E - 1]) or all(
            pt[i] == 0 for i in range(e, E)
        ):
            pass
        leftover = total - sum(s[2] for core in segs for s in core)
        if leftover == 0:
            return nt_try, segs
    return None


def kernel(x, num_tokens_per_expert, w1, w2, w3):
    from concourse.bass_utils import run_bass_kernel_spmd

    x = np.asarray(x)
    counts = [int(v) for v in np.asarray(num_tokens_per_expert)]
    w1 = np.asarray(w1)
    w2 = np.asarray(w2)
    w3 = np.asarray(w3)
    T, E = x.shape[0], len(counts)
    assert E == NCORES
    starts = np.concatenate([[0], np.cumsum(counts)])[:E].astype(np.int64)

    plan = _assign(counts)
    if plan is None:
        # fallback: expert-parallel (1 segment per core), padded to max tiles
        pt = [max(1, math.ceil(c / MT)) if c > 0 else 0 for c in counts]
        nt = max(pt)
        segs = [[(e, 0, pt[e])] if pt[e] else [] for e in range(NCORES)]
    nt, segs = (plan if plan is not None else (nt, segs))
    nc = _get_program(nt)
    PAD_T = nt * MT

    w1b = w1.astype(ml_dtypes.bfloat16)
    w2b = w2.astype(ml_dtypes.bfloat16)
    w3b = w3.astype(ml_dtypes.bfloat16)
    xT = np.ascontiguousarray(x.T).astype(ml_dtypes.bfloat16)  # [D, T]

    in_maps = []
    placements = []  # per core: list of (slot, src_lo, n_rows)
    for c in range(NCORES):
        xpt = np.zeros((D, PAD_T), dtype=ml_dtypes.bfloat16)
        place = []
        slot = 0
        cs = segs[c]
        ta = cs[0][2] if cs else 0
        exps = [s[0] for s in cs]
        ea = exps[0] if exps else 0
        eb = exps[1] if len(exps) > 1 else ea
        for (e, tile_lo, ntk) in cs:
            src_lo = int(starts[e]) + tile_lo * MT
            src_hi = min(int(starts[e]) + counts[e], src_lo + ntk * MT)
            nrow = src_hi - src_lo
            xpt[:, slot * MT: slot * MT + nrow] = xT[:, src_lo:src_hi]
            place.append((slot, src_lo, nrow))
            slot += ntk
        placements.append(place)
        in_maps.append({
            "xpt": xpt,
            "wa1": np.ascontiguousarray(w1b[ea]),
            "wa2": np.ascontiguousarray(w2b[ea]),
            "wa3": np.ascontiguousarray(w3b[ea]),
            "wb1": np.ascontiguousarray(w1b[eb]),
            "wb2": np.ascontiguousarray(w2b[eb]),
            "wb3": np.ascontiguousarray(w3b[eb]),
            "meta": np.array([[ta]], dtype=np.int32),
        })

    trace = bool(int(os.environ.get("KERNEL_TRACE", "0")))
    try:
        res = run_bass_kernel_spmd(nc, in_maps, core_ids=list(range(NCORES)),
                                   trace=trace)
    except ModuleNotFoundError:
        res = run_bass_kernel_spmd(nc, in_maps, core_ids=list(range(NCORES)),
                                   trace=False)
    kernel.last_results = res

    out = np.empty((T, D), dtype=np.float32)
    for c in range(NCORES):
        o = res.results[c]["out"]
        for (slot, src_lo, nrow) in placements[c]:
            out[src_lo:src_lo + nrow] = o[slot * MT: slot * MT + nrow]
    return out
